# revision 1
# baseline (speedup 1.0000x reference)
"""Trainium2 Bass kernel for nn_Decoder (ragged_sequence).

Computes: sigmas = x@W_sig + b_sig; h = swish(x@W1 + b1); y = h@W2 + b2;
then per-segment gaussian smoothing (5 equal segments of 20000, window
10000, sigma ~ 200) of y, concatenated.

Strategy (8 NeuronCores, SPMD, full I/O):
  - Host computes the tiny parts (sigmas, h, gaussian windows) in numpy.
  - Output vector N=100000 is sharded over 8 cores (12500 each, rounded
    out to 99 blocks of 128). Each core GEMMs its slice of W2 columns
    (plus +-6 blocks of halo, zero-padded where the halo crosses a
    segment/global boundary) against h, producing y in a [128, cols]
    block layout (partition = position % 128).
  - The gaussian conv is applied as 13 shifted Toeplitz 128x128 matmuls
    accumulated in PSUM (window truncated to +-895 taps; sigma~200 so
    truncated relative error ~4e-4). Segment boundaries inside a core's
    range are handled with left/right input masks + left/right tap
    tiles + an output select -- all host-built data, so one uniform
    SPMD program serves all cores.
"""

import os
from contextlib import ExitStack

import numpy as np

import ml_dtypes

import concourse.bass as bass
import concourse.mybir as mybir
import concourse.tile as tile
from concourse import bacc
from concourse.bass_utils import run_bass_kernel_spmd

# ---------------------------------------------------------------- constants
D = 128
H = 512
N = 100000
NSIG = 5
WIN = 10000          # reference window size
SEGL = 20000         # segment length
NCORES = 8
PER = N // NCORES    # 12500 outputs per core
BLK = 128
OUTB = 99            # output blocks per core (99*128 = 12672 >= 12500 + max misalign 84)
HB = 6               # halo blocks on each side (+-768 positions)
EXTB = OUTB + 2 * HB # 111 ext blocks of y per core
TAPB = 2 * HB + 1    # 13 Toeplitz shift tiles
CHUNKS = [4, 4, 8, 8, 12, 16, 16, 16, 16, 11]  # W2 column-block chunks (sum = EXTB)
KCH = H // BLK       # 4 contraction chunks

assert sum(CHUNKS) == EXTB

BSTART = [(k * PER) // BLK for k in range(NCORES)]

_DT = os.environ.get("BASS_DECODER_DTYPE", "bf16")

_CACHED_NC = {}


def _np_dt(dt):
    return ml_dtypes.bfloat16 if dt == "bf16" else np.float32


def _mybir_dt(dt):
    return mybir.dt.bfloat16 if dt == "bf16" else mybir.dt.float32


# ---------------------------------------------------------------- device IR
def _build_nc(dt: str):
    """Build + compile the SPMD Bass kernel (same program for all cores)."""
    if dt in _CACHED_NC:
        return _CACHED_NC[dt]
    f32 = mybir.dt.float32
    dtc = _mybir_dt(dt)

    nc = bacc.Bacc(
        "TRN2",
        target_bir_lowering=False,
        debug=False,
        enable_asserts=False,
        num_devices=NCORES,
    )
    ins = {}

    def din(name, shape, dtt):
        ins[name] = nc.dram_tensor(name, shape, dtt, kind="ExternalInput").ap()

    din("w2e", [H, EXTB * BLK], dtc)
    din("h", [BLK, KCH], dtc)
    din("b2e", [BLK, EXTB], f32)
    din("tl", [BLK, TAPB * BLK], dtc)
    din("tr", [BLK, TAPB * BLK], dtc)
    din("ml", [BLK, EXTB], f32)
    din("mr", [BLK, EXTB], f32)
    din("oml", [BLK, OUTB], f32)
    din("omr", [BLK, OUTB], f32)
    out_ap = nc.dram_tensor("out", [BLK, OUTB], f32, kind="ExternalOutput").ap()

    with tile.TileContext(nc) as tc:
        with ExitStack() as ctx:
            const = ctx.enter_context(tc.tile_pool(name="const", bufs=1))
            wpool = ctx.enter_context(tc.tile_pool(name="w", bufs=6))
            sbp = ctx.enter_context(tc.tile_pool(name="sb", bufs=1))
            pgp = ctx.enter_context(tc.tile_pool(name="pg", bufs=3, space="PSUM"))
            pcp = ctx.enter_context(tc.tile_pool(name="pc", bufs=1, space="PSUM"))

            # constants go on non-Sync DMA queues so the W2 stream starts
            # immediately
            h_sb = const.tile([BLK, KCH], dtc)
            nc.gpsimd.dma_start(h_sb[:], ins["h"][:])
            b2_sb = const.tile([BLK, EXTB], f32)
            nc.scalar.dma_start(b2_sb[:], ins["b2e"][:])
            ml_sb = const.tile([BLK, EXTB], f32)
            nc.scalar.dma_start(ml_sb[:], ins["ml"][:])
            mr_sb = const.tile([BLK, EXTB], f32)
            nc.scalar.dma_start(mr_sb[:], ins["mr"][:])
            tl_sb = const.tile([BLK, TAPB * BLK], dtc)
            nc.gpsimd.dma_start(tl_sb[:], ins["tl"][:])
            tr_sb = const.tile([BLK, TAPB * BLK], dtc)
            nc.gpsimd.dma_start(tr_sb[:], ins["tr"][:])
            oml_sb = const.tile([BLK, OUTB], f32)
            nc.scalar.dma_start(oml_sb[:], ins["oml"][:])
            omr_sb = const.tile([BLK, OUTB], f32)
            nc.scalar.dma_start(omr_sb[:], ins["omr"][:])

            s_sb = sbp.tile([BLK, EXTB], f32)
            sl_sb = sbp.tile([BLK, EXTB], dtc)
            sr_sb = sbp.tile([BLK, EXTB], dtc)

            # ---- GEMM: y[c*128+q] = sum_k h[k] * W2[k, c*128+q]  (+ b2)
            # four plain 2D DMAs per chunk on one queue sustain ~400 GB/s
            # (a single 3D k-interleaved DMA only reaches ~215 GB/s)
            c0 = 0
            for ci, cb_n in enumerate(CHUNKS):
                wts = []
                for kc in range(KCH):
                    wt = wpool.tile([BLK, cb_n * BLK], dtc, tag=f"w{kc}")
                    nc.sync.dma_start(
                        wt[:],
                        ins["w2e"][
                            kc * BLK : (kc + 1) * BLK,
                            c0 * BLK : (c0 + cb_n) * BLK,
                        ],
                    )
                    wts.append(wt)
                psum_g = pgp.tile([BLK, cb_n], f32, tag="pg")
                for cb in range(cb_n):
                    for kc in range(KCH):
                        nc.tensor.matmul(
                            psum_g[:, cb : cb + 1],
                            lhsT=wts[kc][:, cb * BLK : (cb + 1) * BLK],
                            rhs=h_sb[:, kc : kc + 1],
                            start=(kc == 0),
                            stop=(kc == KCH - 1),
                        )
                # finalize this chunk's S columns (bias + boundary masks)
                sl = slice(c0, c0 + cb_n)
                nc.vector.tensor_add(s_sb[:, sl], psum_g[:], b2_sb[:, sl])
                nc.vector.tensor_mul(sl_sb[:, sl], s_sb[:, sl], ml_sb[:, sl])
                nc.vector.tensor_mul(sr_sb[:, sl], s_sb[:, sl], mr_sb[:, sl])
                c0 += cb_n

            # ---- conv: out[q,c] = sum_e sum_p T[e][p,q] * S[p, c+HB+e]
            pA = pcp.tile([BLK, OUTB], f32, tag="pA")
            pB = pcp.tile([BLK, OUTB], f32, tag="pB")
            for ei in range(TAPB):
                nc.tensor.matmul(
                    pA[:, :],
                    lhsT=tl_sb[:, ei * BLK : (ei + 1) * BLK],
                    rhs=sl_sb[:, ei : ei + OUTB],
                    start=(ei == 0),
                    stop=(ei == TAPB - 1),
                )
            for ei in range(TAPB):
                nc.tensor.matmul(
                    pB[:, :],
                    lhsT=tr_sb[:, ei * BLK : (ei + 1) * BLK],
                    rhs=sr_sb[:, ei : ei + OUTB],
                    start=(ei == 0),
                    stop=(ei == TAPB - 1),
                )

            t1 = sbp.tile([BLK, OUTB], f32)
            nc.vector.tensor_mul(t1[:], pA[:], oml_sb[:])
            t2 = sbp.tile([BLK, OUTB], f32)
            nc.vector.tensor_mul(t2[:], pB[:], omr_sb[:])
            o_sb = sbp.tile([BLK, OUTB], f32)
            nc.vector.tensor_add(o_sb[:], t1[:], t2[:])
            nc.sync.dma_start(out_ap[:], o_sb[:])

    nc.compile()
    _CACHED_NC[dt] = nc
    return nc


# ---------------------------------------------------------------- host prep
def _prep_inputs(x, W_sig, b_sig, W1, b1, W2, b2, dt: str):
    npdt = _np_dt(dt)
    f64 = np.float64

    # tiny head + MLP hidden layer on host
    sig = x.astype(f64) @ W_sig.astype(f64) + b_sig.astype(f64)       # [5]
    pre = x.astype(f64) @ W1.astype(f64) + b1.astype(f64)             # [512]
    h = pre / (1.0 + np.exp(-pre))                                    # swish

    # normalized gaussian taps per segment: G_s(m) = exp(-m^2/2s^2)/Z_s
    # (Z over the full reference window t=0..9999 centered at 5000)
    t = np.arange(WIN, dtype=f64)
    Z = np.exp(-((t[None, :] - WIN / 2) ** 2) / (2 * sig[:, None] ** 2)).sum(axis=1)

    p = np.arange(BLK)[:, None]
    q = np.arange(BLK)[None, :]
    e = np.arange(-HB, HB + 1)[:, None, None]
    m = e * BLK + p[None] - q[None] + 1                               # [13,128,128]
    tiles = []
    for s in range(NSIG):
        g = np.exp(-(m.astype(f64) ** 2) / (2 * sig[s] ** 2)) / Z[s]
        tiles.append(np.ascontiguousarray(g.transpose(1, 0, 2)).reshape(BLK, -1))

    h_in = np.ascontiguousarray(h.reshape(KCH, BLK).T).astype(npdt)

    in_maps = []
    meta = []
    for k in range(NCORES):
        lo = (BSTART[k] - HB) * BLK
        hi = lo + EXTB * BLK
        out0 = BSTART[k] * BLK
        glo, ghi = max(lo, 0), min(hi, N)

        w2e = np.zeros((H, EXTB * BLK), dtype=npdt)
        w2e[:, glo - lo : ghi - lo] = W2[:, glo:ghi].astype(npdt)
        b2p = np.zeros(EXTB * BLK, dtype=np.float32)
        b2p[glo - lo : ghi - lo] = b2[glo:ghi]
        b2e = np.ascontiguousarray(b2p.reshape(EXTB, BLK).T)

        B = None
        for b in range(SEGL, N, SEGL):
            if lo < b < hi:
                B = b
        ext_pos = lo + np.arange(EXTB)[None, :] * BLK + np.arange(BLK)[:, None]
        out_pos = out0 + np.arange(OUTB)[None, :] * BLK + np.arange(BLK)[:, None]
        if B is None:
            seg = min(out0 // SEGL, NSIG - 1)
            tl = tr = tiles[seg]
            ml = np.ones((BLK, EXTB), np.float32)
            mr = np.zeros((BLK, EXTB), np.float32)
            oml = np.ones((BLK, OUTB), np.float32)
            omr = np.zeros((BLK, OUTB), np.float32)
        else:
            tl = tiles[B // SEGL - 1]
            tr = tiles[B // SEGL]
            ml = (ext_pos < B).astype(np.float32)
            mr = (ext_pos >= B).astype(np.float32)
            oml = (out_pos < B).astype(np.float32)
            omr = (out_pos >= B).astype(np.float32)

        in_maps.append(
            {
                "w2e": w2e,
                "h": h_in,
                "b2e": b2e,
                "tl": np.ascontiguousarray(tl).astype(npdt),
                "tr": np.ascontiguousarray(tr).astype(npdt),
                "ml": ml,
                "mr": mr,
                "oml": oml,
                "omr": omr,
            }
        )
        meta.append((out0, k * PER - out0))
    return in_maps, meta


def _assemble(results, meta):
    full = np.empty(N, dtype=np.float32)
    for k in range(NCORES):
        arr = results[k]["out"]                         # [128, OUTB]
        flat = np.ascontiguousarray(arr.T).reshape(-1)  # pos out0 + i
        off = meta[k][1]
        full[k * PER : (k + 1) * PER] = flat[off : off + PER]
    return full


def run_with_results(inputs: dict, dt: str | None = None, trace: bool = False):
    dt = dt or _DT
    args = {k: np.asarray(v, dtype=np.float32) for k, v in inputs.items()}
    in_maps, meta = _prep_inputs(
        args["x"], args["W_sig"], args["b_sig"], args["W1"], args["b1"],
        args["W2"], args["b2"], dt,
    )
    nc = _build_nc(dt)
    res = run_bass_kernel_spmd(
        nc, in_maps, core_ids=list(range(NCORES)), trace=trace
    )
    return _assemble(res.results, meta), res


def kernel(**inputs) -> np.ndarray:
    out, _ = run_with_results(inputs)
    return out



# revision 3
# speedup vs baseline: 1.2280x; 1.2280x over previous
"""Trainium2 Bass kernel for nn_Decoder (ragged_sequence).

Computes: sigmas = x@W_sig + b_sig; h = swish(x@W1 + b1); y = h@W2 + b2;
then per-segment gaussian smoothing (5 equal segments of 20000, window
10000, sigma ~ 200) of y, concatenated.

Strategy (8 NeuronCores, SPMD, full I/O):
  - Host computes the tiny parts (sigmas, h, gaussian windows) in numpy.
  - Output vector N=100000 is sharded over 8 cores (12500 each, rounded
    out to 99 blocks of 128). Each core GEMMs its slice of W2 columns
    (plus +-5 blocks of halo, zero-padded at the global edges) against
    h, producing y in a [128, cols] block layout (partition = pos%128).
  - Mixed precision, h-magnitude sorted: the 128 rows of W2 with the
    largest |h_k| are kept in fp16; the remaining 384 rows are quantized
    to fp8 e3m4 (x32 scale folded into the h operand). Quantization
    noise ~ sqrt(tail h^2 fraction)*1.3% ~ 0.45 %.
  - W2 slices are pre-tiled on the host into [128, X] DRAM tensors so
    each chunk is ONE contiguous-row 2D DMA descriptor (descriptor
    issue on the Sync engine costs ~630ns each; the baseline's 40
    descriptors serialized the stream). fp16 stream on the sync queue,
    fp8 stream on the scalar queue, constants on gpsimd.
  - The gaussian conv is applied as 11 shifted Toeplitz 128x128 matmuls
    accumulated in PSUM (window truncated to +-~767 taps, rel err
    ~4e-4). Segment boundaries are handled with left/right input masks
    + left/right tap tiles + an output select (host-built data, one
    uniform SPMD program for all cores).
"""

import os
from contextlib import ExitStack

import numpy as np

import ml_dtypes

import concourse.bass as bass
import concourse.mybir as mybir
import concourse.tile as tile
from concourse import bacc
from concourse.bass_utils import run_bass_kernel_spmd

# ---------------------------------------------------------------- constants
D = 128
H = 512
N = 100000
NSIG = 5
WIN = 10000          # reference window size
SEGL = 20000         # segment length
NCORES = 8
PER = N // NCORES    # 12500 outputs per core
BLK = 128
OUTB = 99            # output blocks per core (99*128 = 12672 >= 12500 + 84)
HB = 5               # halo blocks each side (+-640; sigma~200 -> err ~4e-4)
EXTB = OUTB + 2 * HB # 109 ext blocks of y per core
TAPB = 2 * HB + 1    # 11 Toeplitz shift tiles
CHUNKS = [16, 16, 16, 16, 16, 16, 13]  # W2 column-block chunks (sum = EXTB)
RB = 128             # rows (by |h| desc) kept in fp16
KQ = (H - RB) // BLK # 3 fp8 k-chunks
QS = 32.0            # fp8 weight scale (folded into the fp8-side h operand)

assert sum(CHUNKS) == EXTB

BSTART = [(k * PER) // BLK for k in range(NCORES)]

F8 = ml_dtypes.float8_e3m4

_CACHED_NC = {}


# ---------------------------------------------------------------- device IR
def _build_nc():
    """Build + compile the SPMD Bass kernel (same program for all cores)."""
    if "nc" in _CACHED_NC:
        return _CACHED_NC["nc"]
    f32 = mybir.dt.float32
    f16 = mybir.dt.float16
    f8e3 = mybir.dt.float8e3

    nc = bacc.Bacc(
        "TRN2",
        target_bir_lowering=False,
        debug=False,
        enable_asserts=False,
        num_devices=NCORES,
    )
    ins = {}

    def din(name, shape, dtt):
        ins[name] = nc.dram_tensor(name, shape, dtt, kind="ExternalInput").ap()

    din("wb", [BLK, EXTB * BLK], f16)           # fp16 W2 rows, col-tiled
    din("wq", [BLK, KQ * EXTB * BLK], f8e3)     # fp8 W2 rows, chunk-major
    din("hb", [BLK, 1], f16)
    din("hq", [BLK, KQ], f16)
    din("b2e", [BLK, EXTB], f32)
    din("tl", [BLK, TAPB * BLK], f16)
    din("tr", [BLK, TAPB * BLK], f16)
    din("ml", [BLK, EXTB], f32)
    din("mr", [BLK, EXTB], f32)
    din("oml", [BLK, OUTB], f32)
    din("omr", [BLK, OUTB], f32)
    out_ap = nc.dram_tensor("out", [BLK, OUTB], f32, kind="ExternalOutput").ap()

    with tile.TileContext(nc) as tc:
        with ExitStack() as ctx:
            const = ctx.enter_context(tc.tile_pool(name="const", bufs=1))
            wpool = ctx.enter_context(tc.tile_pool(name="w", bufs=4))
            sbp = ctx.enter_context(tc.tile_pool(name="sb", bufs=1))
            pgp = ctx.enter_context(tc.tile_pool(name="pg", bufs=3, space="PSUM"))
            pcp = ctx.enter_context(tc.tile_pool(name="pc", bufs=1, space="PSUM"))

            # constants go on the gpsimd DMA queue so the W2 streams
            # (sync + scalar queues) start immediately; h first, it
            # gates the first matmul
            hb_sb = const.tile([BLK, 1], f16)
            nc.gpsimd.dma_start(hb_sb[:], ins["hb"][:])
            hq_sb = const.tile([BLK, KQ], f16)
            nc.gpsimd.dma_start(hq_sb[:], ins["hq"][:])
            b2_sb = const.tile([BLK, EXTB], f32)
            nc.gpsimd.dma_start(b2_sb[:], ins["b2e"][:])
            ml_sb = const.tile([BLK, EXTB], f32)
            nc.gpsimd.dma_start(ml_sb[:], ins["ml"][:])
            mr_sb = const.tile([BLK, EXTB], f32)
            nc.gpsimd.dma_start(mr_sb[:], ins["mr"][:])
            tl_sb = const.tile([BLK, TAPB * BLK], f16)
            nc.gpsimd.dma_start(tl_sb[:], ins["tl"][:])
            tr_sb = const.tile([BLK, TAPB * BLK], f16)
            nc.gpsimd.dma_start(tr_sb[:], ins["tr"][:])
            oml_sb = const.tile([BLK, OUTB], f32)
            nc.gpsimd.dma_start(oml_sb[:], ins["oml"][:])
            omr_sb = const.tile([BLK, OUTB], f32)
            nc.gpsimd.dma_start(omr_sb[:], ins["omr"][:])

            s_sb = sbp.tile([BLK, EXTB], f32)
            sl_sb = sbp.tile([BLK, EXTB], f16)
            sr_sb = sbp.tile([BLK, EXTB], f16)

            # ---- GEMM: y[c*128+q] = sum_k h[k] * W2[k, c*128+q]  (+ b2)
            c0 = 0
            for ci, cbn in enumerate(CHUNKS):
                wbt = wpool.tile([BLK, cbn * BLK], f16, tag="wb")
                nc.sync.dma_start(
                    wbt[:], ins["wb"][:, c0 * BLK : (c0 + cbn) * BLK]
                )
                wqt = wpool.tile([BLK, KQ * cbn * BLK], f8e3, tag="wq")
                nc.scalar.dma_start(
                    wqt[:],
                    ins["wq"][:, KQ * c0 * BLK : KQ * (c0 + cbn) * BLK],
                )
                psum_g = pgp.tile([BLK, cbn], f32, tag="pg")
                for cb in range(cbn):
                    nc.tensor.matmul(
                        psum_g[:, cb : cb + 1],
                        lhsT=wbt[:, cb * BLK : (cb + 1) * BLK],
                        rhs=hb_sb[:],
                        start=True,
                        stop=False,
                    )
                    for kc in range(KQ):
                        nc.tensor.matmul(
                            psum_g[:, cb : cb + 1],
                            lhsT=wqt[:, (kc * cbn + cb) * BLK : (kc * cbn + cb + 1) * BLK],
                            rhs=hq_sb[:, kc : kc + 1],
                            start=False,
                            stop=(kc == KQ - 1),
                        )
                # finalize this chunk's S columns (bias + boundary masks)
                sl = slice(c0, c0 + cbn)
                nc.vector.tensor_add(s_sb[:, sl], psum_g[:], b2_sb[:, sl])
                nc.vector.tensor_mul(sl_sb[:, sl], s_sb[:, sl], ml_sb[:, sl])
                nc.vector.tensor_mul(sr_sb[:, sl], s_sb[:, sl], mr_sb[:, sl])
                c0 += cbn

            # ---- conv: out[q,c] = sum_e sum_p T[e][p,q] * S[p, c+HB+e]
            pA = pcp.tile([BLK, OUTB], f32, tag="pA")
            pB = pcp.tile([BLK, OUTB], f32, tag="pB")
            for ei in range(TAPB):
                nc.tensor.matmul(
                    pA[:, :],
                    lhsT=tl_sb[:, ei * BLK : (ei + 1) * BLK],
                    rhs=sl_sb[:, ei : ei + OUTB],
                    start=(ei == 0),
                    stop=(ei == TAPB - 1),
                )
            for ei in range(TAPB):
                nc.tensor.matmul(
                    pB[:, :],
                    lhsT=tr_sb[:, ei * BLK : (ei + 1) * BLK],
                    rhs=sr_sb[:, ei : ei + OUTB],
                    start=(ei == 0),
                    stop=(ei == TAPB - 1),
                )

            t1 = sbp.tile([BLK, OUTB], f32)
            nc.vector.tensor_mul(t1[:], pA[:], oml_sb[:])
            t2 = sbp.tile([BLK, OUTB], f32)
            nc.vector.tensor_mul(t2[:], pB[:], omr_sb[:])
            o_sb = sbp.tile([BLK, OUTB], f32)
            nc.vector.tensor_add(o_sb[:], t1[:], t2[:])
            nc.gpsimd.dma_start(out_ap[:], o_sb[:])

    nc.compile()
    _CACHED_NC["nc"] = nc
    return nc


# ---------------------------------------------------------------- host prep
def _prep_inputs(x, W_sig, b_sig, W1, b1, W2, b2):
    f64 = np.float64

    # tiny head + MLP hidden layer on host
    sig = x.astype(f64) @ W_sig.astype(f64) + b_sig.astype(f64)       # [5]
    pre = x.astype(f64) @ W1.astype(f64) + b1.astype(f64)             # [512]
    h = pre / (1.0 + np.exp(-pre))                                    # swish

    # mixed-precision split: biggest |h| rows stay fp16, rest fp8 e3m4
    ordr = np.argsort(-np.abs(h), kind="stable")
    hb_in = np.ascontiguousarray(h[ordr[:RB]].reshape(RB, 1)).astype(np.float16)
    hq_in = np.ascontiguousarray(
        (h[ordr[RB:]] / QS).reshape(KQ, BLK).T
    ).astype(np.float16)
    W2b = np.ascontiguousarray(W2[ordr[:RB], :]).astype(np.float16)   # [128,N]
    W2q = (np.ascontiguousarray(W2[ordr[RB:], :]) * np.float32(QS)).astype(F8)

    # normalized gaussian taps per segment: G_s(m) = exp(-m^2/2s^2)/Z_s
    # (Z over the full reference window t=0..9999 centered at 5000)
    t = np.arange(WIN, dtype=f64)
    Z = np.exp(-((t[None, :] - WIN / 2) ** 2) / (2 * sig[:, None] ** 2)).sum(axis=1)

    p = np.arange(BLK)[:, None]
    q = np.arange(BLK)[None, :]
    e = np.arange(-HB, HB + 1)[:, None, None]
    m = e * BLK + p[None] - q[None] + 1                               # [11,128,128]
    tiles = []
    for s in range(NSIG):
        g = np.exp(-(m.astype(f64) ** 2) / (2 * sig[s] ** 2)) / Z[s]
        tiles.append(
            np.ascontiguousarray(g.transpose(1, 0, 2)).reshape(BLK, -1)
        )

    in_maps = []
    meta = []
    for k in range(NCORES):
        lo = (BSTART[k] - HB) * BLK
        hi = lo + EXTB * BLK
        out0 = BSTART[k] * BLK
        glo, ghi = max(lo, 0), min(hi, N)

        wb = np.zeros((BLK, EXTB * BLK), dtype=np.float16)
        wb[:, glo - lo : ghi - lo] = W2b[:, glo:ghi]
        wqf = np.zeros((KQ * BLK, EXTB * BLK), dtype=F8)
        wqf[:, glo - lo : ghi - lo] = W2q[:, glo:ghi]
        # chunk-major fp8 layout: per chunk [kc][cb][q] so each chunk is
        # one contiguous-row DMA descriptor
        parts = []
        c0 = 0
        for cbn in CHUNKS:
            blockcols = wqf[:, c0 * BLK : (c0 + cbn) * BLK]
            parts.append(
                blockcols.reshape(KQ, BLK, cbn * BLK)
                .transpose(1, 0, 2)
                .reshape(BLK, KQ * cbn * BLK)
            )
            c0 += cbn
        wq = np.ascontiguousarray(np.concatenate(parts, axis=1))

        b2p = np.zeros(EXTB * BLK, dtype=np.float32)
        b2p[glo - lo : ghi - lo] = b2[glo:ghi]
        b2e = np.ascontiguousarray(b2p.reshape(EXTB, BLK).T)

        B = None
        for b in range(SEGL, N, SEGL):
            if lo < b < hi:
                B = b
        ext_pos = lo + np.arange(EXTB)[None, :] * BLK + np.arange(BLK)[:, None]
        out_pos = out0 + np.arange(OUTB)[None, :] * BLK + np.arange(BLK)[:, None]
        if B is None:
            seg = min(out0 // SEGL, NSIG - 1)
            tl = tr = tiles[seg]
            ml = np.ones((BLK, EXTB), np.float32)
            mr = np.zeros((BLK, EXTB), np.float32)
            oml = np.ones((BLK, OUTB), np.float32)
            omr = np.zeros((BLK, OUTB), np.float32)
        else:
            tl = tiles[B // SEGL - 1]
            tr = tiles[B // SEGL]
            ml = (ext_pos < B).astype(np.float32)
            mr = (ext_pos >= B).astype(np.float32)
            oml = (out_pos < B).astype(np.float32)
            omr = (out_pos >= B).astype(np.float32)

        in_maps.append(
            {
                "wb": wb,
                "wq": wq,
                "hb": hb_in,
                "hq": hq_in,
                "b2e": b2e,
                "tl": np.ascontiguousarray(tl).astype(np.float16),
                "tr": np.ascontiguousarray(tr).astype(np.float16),
                "ml": ml,
                "mr": mr,
                "oml": oml,
                "omr": omr,
            }
        )
        meta.append((out0, k * PER - out0))
    return in_maps, meta


def _assemble(results, meta):
    full = np.empty(N, dtype=np.float32)
    for k in range(NCORES):
        arr = results[k]["out"]                         # [128, OUTB]
        flat = np.ascontiguousarray(arr.T).reshape(-1)  # pos out0 + i
        off = meta[k][1]
        full[k * PER : (k + 1) * PER] = flat[off : off + PER]
    return full


def run_with_results(inputs: dict, dt: str | None = None, trace: bool = False):
    args = {k: np.asarray(v, dtype=np.float32) for k, v in inputs.items()}
    in_maps, meta = _prep_inputs(
        args["x"], args["W_sig"], args["b_sig"], args["W1"], args["b1"],
        args["W2"], args["b2"],
    )
    nc = _build_nc()
    res = run_bass_kernel_spmd(
        nc, in_maps, core_ids=list(range(NCORES)), trace=trace
    )
    return _assemble(res.results, meta), res


def kernel(**inputs) -> np.ndarray:
    out, _ = run_with_results(inputs)
    return out


# revision 6
# speedup vs baseline: 1.3535x; 1.1022x over previous
"""Trainium2 Bass kernel for nn_Decoder (ragged_sequence).

Computes: sigmas = x@W_sig + b_sig; h = swish(x@W1 + b1); y = h@W2 + b2;
then per-segment gaussian smoothing (5 equal segments of 20000, window
10000, sigma ~ 200) of y, concatenated.

Strategy (8 NeuronCores, SPMD, full I/O):
  - Host computes the tiny parts (sigmas, h, gaussian windows) in numpy.
  - Output vector N=100000 is sharded over 8 cores (12500 each, rounded
    out to 99 blocks of 128). Each core GEMMs its slice of W2 columns
    (plus +-5 blocks of halo, zero-padded at the global edges) against
    h, producing y in a [128, cols] block layout (partition = pos%128).
  - Mixed precision, h-magnitude sorted: the 128 rows of W2 with the
    largest |h_k| are kept in fp16; the remaining 384 rows are quantized
    to fp8 e3m4 (x32 scale folded into the h operand). Quantization
    noise ~ sqrt(tail h^2 fraction)*1.3% ~ 0.45 %.
  - W2 slices are pre-tiled on the host into [128, X] DRAM tensors so
    each chunk is ONE contiguous-row 2D DMA descriptor (descriptor
    issue on the Sync engine costs ~630ns each; the baseline's 40
    descriptors serialized the stream). fp16 stream on the sync queue,
    fp8 stream on the scalar queue, constants on gpsimd.
  - The gaussian conv is applied as 11 shifted Toeplitz 128x128 matmuls
    accumulated in PSUM (window truncated to +-~767 taps, rel err
    ~4e-4). Segment boundaries are handled with left/right input masks
    + left/right tap tiles + an output select (host-built data, one
    uniform SPMD program for all cores).
"""

import os
from contextlib import ExitStack

import numpy as np

import ml_dtypes

import concourse.bass as bass
import concourse.mybir as mybir
import concourse.tile as tile
from concourse import bacc
from concourse.bass_utils import run_bass_kernel_spmd

# ---------------------------------------------------------------- constants
D = 128
H = 512
N = 100000
NSIG = 5
WIN = 10000          # reference window size
SEGL = 20000         # segment length
NCORES = 8
PER = N // NCORES    # 12500 outputs per core
BLK = 128
OUTB = 99            # output blocks per core (99*128 = 12672 >= 12500 + 84)
HB = 5               # halo blocks each side (+-640; sigma~200 -> err ~4e-4)
EXTB = OUTB + 2 * HB # 109 ext blocks of y per core
TAPB = 2 * HB + 1    # 11 Toeplitz shift tiles
CHUNKS = [16, 16, 16, 16, 16, 16, 13]  # W2 column-block chunks (sum = EXTB)
RB = 128             # rows (by |h| desc) kept in fp16
KQ = (H - RB) // BLK # 3 fp8 k-chunks
QS = 32.0            # fp8 weight scale (folded into the fp8-side h operand)

assert sum(CHUNKS) == EXTB

BSTART = [(k * PER) // BLK for k in range(NCORES)]

F8 = ml_dtypes.float8_e3m4

_CACHED_NC = {}


# ---------------------------------------------------------------- device IR
def _build_nc():
    """Build + compile the SPMD Bass kernel (same program for all cores)."""
    if "nc" in _CACHED_NC:
        return _CACHED_NC["nc"]
    f32 = mybir.dt.float32
    f16 = mybir.dt.float16
    f8e3 = mybir.dt.float8e3

    nc = bacc.Bacc(
        "TRN2",
        target_bir_lowering=False,
        debug=False,
        enable_asserts=False,
        num_devices=NCORES,
    )
    ins = {}

    def din(name, shape, dtt):
        ins[name] = nc.dram_tensor(name, shape, dtt, kind="ExternalInput").ap()

    din("wb", [BLK, EXTB * BLK], f16)           # fp16 W2 rows, col-tiled
    din("wq", [BLK, KQ * EXTB * BLK], f8e3)     # fp8 W2 rows, chunk-major
    din("hb", [BLK, 1], f16)
    din("hq", [BLK, KQ], f16)
    din("b2e", [BLK, EXTB], f32)
    din("tl", [BLK, TAPB * BLK], f16)
    din("tr", [BLK, TAPB * BLK], f16)
    din("ml", [BLK, EXTB], f32)
    din("mr", [BLK, EXTB], f32)
    din("oml", [BLK, OUTB], f32)
    din("omr", [BLK, OUTB], f32)
    out_ap = nc.dram_tensor("out", [BLK, OUTB], f32, kind="ExternalOutput").ap()

    with tile.TileContext(nc) as tc:
        with ExitStack() as ctx:
            const = ctx.enter_context(tc.tile_pool(name="const", bufs=1))
            wpool = ctx.enter_context(tc.tile_pool(name="w", bufs=6))
            sbp = ctx.enter_context(tc.tile_pool(name="sb", bufs=1))
            pgp = ctx.enter_context(tc.tile_pool(name="pg", bufs=3, space="PSUM"))
            pcp = ctx.enter_context(tc.tile_pool(name="pc", bufs=1, space="PSUM"))

            # All DMAs ride the two hardware queues (sync + scalar); the
            # gpsimd software queue only sustains ~27 GB/s with ~12us
            # first-byte latency.  Small constants are ordered around the
            # W2 chunk stream by when they are needed: h gates the first
            # matmul, b2/ml/mr the first chunk epilogue, taps + output
            # masks only the final conv.
            hb_sb = const.tile([BLK, 1], f16)
            nc.sync.dma_start(hb_sb[:], ins["hb"][:])
            hq_sb = const.tile([BLK, KQ], f16)
            nc.sync.dma_start(hq_sb[:], ins["hq"][:])
            b2_sb = const.tile([BLK, EXTB], f32)
            nc.sync.dma_start(b2_sb[:], ins["b2e"][:])
            ml_sb = const.tile([BLK, EXTB], f32)
            nc.scalar.dma_start(ml_sb[:], ins["ml"][:])
            mr_sb = const.tile([BLK, EXTB], f32)
            nc.scalar.dma_start(mr_sb[:], ins["mr"][:])
            tl_sb = const.tile([BLK, TAPB * BLK], f16)
            tr_sb = const.tile([BLK, TAPB * BLK], f16)
            oml_sb = const.tile([BLK, OUTB], f32)
            omr_sb = const.tile([BLK, OUTB], f32)

            s_sb = sbp.tile([BLK, EXTB], f32)
            sl_sb = sbp.tile([BLK, EXTB], f16)
            sr_sb = sbp.tile([BLK, EXTB], f16)

            # ---- GEMM: y[c*128+q] = sum_k h[k] * W2[k, c*128+q]  (+ b2)
            c0 = 0
            for ci, cbn in enumerate(CHUNKS):
                wbt = wpool.tile([BLK, cbn * BLK], f16, tag="wb")
                nc.sync.dma_start(
                    wbt[:], ins["wb"][:, c0 * BLK : (c0 + cbn) * BLK]
                )
                wqt = wpool.tile([BLK, KQ * cbn * BLK], f8e3, tag="wq")
                nc.scalar.dma_start(
                    wqt[:],
                    ins["wq"][:, KQ * c0 * BLK : KQ * (c0 + cbn) * BLK],
                )
                # interleave the conv constants into the hardware-queue
                # stream so they arrive well before the conv starts
                if ci == 2:
                    nc.sync.dma_start(tl_sb[:], ins["tl"][:])
                elif ci == 3:
                    nc.sync.dma_start(tr_sb[:], ins["tr"][:])
                elif ci == 4:
                    nc.scalar.dma_start(oml_sb[:], ins["oml"][:])
                    nc.scalar.dma_start(omr_sb[:], ins["omr"][:])
                psum_g = pgp.tile([BLK, cbn], f32, tag="pg")
                for cb in range(cbn):
                    nc.tensor.matmul(
                        psum_g[:, cb : cb + 1],
                        lhsT=wbt[:, cb * BLK : (cb + 1) * BLK],
                        rhs=hb_sb[:],
                        start=True,
                        stop=False,
                    )
                    for kc in range(KQ):
                        nc.tensor.matmul(
                            psum_g[:, cb : cb + 1],
                            lhsT=wqt[:, (kc * cbn + cb) * BLK : (kc * cbn + cb + 1) * BLK],
                            rhs=hq_sb[:, kc : kc + 1],
                            start=False,
                            stop=(kc == KQ - 1),
                        )
                # finalize this chunk's S columns (bias + boundary masks)
                sl = slice(c0, c0 + cbn)
                nc.vector.tensor_add(s_sb[:, sl], psum_g[:], b2_sb[:, sl])
                nc.vector.tensor_mul(sl_sb[:, sl], s_sb[:, sl], ml_sb[:, sl])
                nc.vector.tensor_mul(sr_sb[:, sl], s_sb[:, sl], mr_sb[:, sl])
                c0 += cbn

            # ---- conv: out[q,c] = sum_e sum_p T[e][p,q] * S[p, c+HB+e]
            pA = pcp.tile([BLK, OUTB], f32, tag="pA")
            pB = pcp.tile([BLK, OUTB], f32, tag="pB")
            for ei in range(TAPB):
                nc.tensor.matmul(
                    pA[:, :],
                    lhsT=tl_sb[:, ei * BLK : (ei + 1) * BLK],
                    rhs=sl_sb[:, ei : ei + OUTB],
                    start=(ei == 0),
                    stop=(ei == TAPB - 1),
                )
            for ei in range(TAPB):
                nc.tensor.matmul(
                    pB[:, :],
                    lhsT=tr_sb[:, ei * BLK : (ei + 1) * BLK],
                    rhs=sr_sb[:, ei : ei + OUTB],
                    start=(ei == 0),
                    stop=(ei == TAPB - 1),
                )

            t1 = sbp.tile([BLK, OUTB], f32)
            nc.vector.tensor_mul(t1[:], pA[:], oml_sb[:])
            t2 = sbp.tile([BLK, OUTB], f32)
            nc.vector.tensor_mul(t2[:], pB[:], omr_sb[:])
            o_sb = sbp.tile([BLK, OUTB], f32)
            nc.vector.tensor_add(o_sb[:], t1[:], t2[:])
            nc.scalar.dma_start(out_ap[:], o_sb[:])

    nc.compile()
    _CACHED_NC["nc"] = nc
    return nc


# ---------------------------------------------------------------- host prep
def _prep_inputs(x, W_sig, b_sig, W1, b1, W2, b2):
    f64 = np.float64

    # tiny head + MLP hidden layer on host
    sig = x.astype(f64) @ W_sig.astype(f64) + b_sig.astype(f64)       # [5]
    pre = x.astype(f64) @ W1.astype(f64) + b1.astype(f64)             # [512]
    h = pre / (1.0 + np.exp(-pre))                                    # swish

    # mixed-precision split: biggest |h| rows stay fp16, rest fp8 e3m4
    ordr = np.argsort(-np.abs(h), kind="stable")
    hb_in = np.ascontiguousarray(h[ordr[:RB]].reshape(RB, 1)).astype(np.float16)
    hq_in = np.ascontiguousarray(
        (h[ordr[RB:]] / QS).reshape(KQ, BLK).T
    ).astype(np.float16)
    W2b = np.ascontiguousarray(W2[ordr[:RB], :]).astype(np.float16)   # [128,N]
    W2q = (np.ascontiguousarray(W2[ordr[RB:], :]) * np.float32(QS)).astype(F8)

    # normalized gaussian taps per segment: G_s(m) = exp(-m^2/2s^2)/Z_s
    # (Z over the full reference window t=0..9999 centered at 5000)
    t = np.arange(WIN, dtype=f64)
    Z = np.exp(-((t[None, :] - WIN / 2) ** 2) / (2 * sig[:, None] ** 2)).sum(axis=1)

    p = np.arange(BLK)[:, None]
    q = np.arange(BLK)[None, :]
    e = np.arange(-HB, HB + 1)[:, None, None]
    m = e * BLK + p[None] - q[None] + 1                               # [11,128,128]
    tiles = []
    for s in range(NSIG):
        g = np.exp(-(m.astype(f64) ** 2) / (2 * sig[s] ** 2)) / Z[s]
        tiles.append(
            np.ascontiguousarray(g.transpose(1, 0, 2)).reshape(BLK, -1)
        )

    in_maps = []
    meta = []
    for k in range(NCORES):
        lo = (BSTART[k] - HB) * BLK
        hi = lo + EXTB * BLK
        out0 = BSTART[k] * BLK
        glo, ghi = max(lo, 0), min(hi, N)

        wb = np.zeros((BLK, EXTB * BLK), dtype=np.float16)
        wb[:, glo - lo : ghi - lo] = W2b[:, glo:ghi]
        wqf = np.zeros((KQ * BLK, EXTB * BLK), dtype=F8)
        wqf[:, glo - lo : ghi - lo] = W2q[:, glo:ghi]
        # chunk-major fp8 layout: per chunk [kc][cb][q] so each chunk is
        # one contiguous-row DMA descriptor
        parts = []
        c0 = 0
        for cbn in CHUNKS:
            blockcols = wqf[:, c0 * BLK : (c0 + cbn) * BLK]
            parts.append(
                blockcols.reshape(KQ, BLK, cbn * BLK)
                .transpose(1, 0, 2)
                .reshape(BLK, KQ * cbn * BLK)
            )
            c0 += cbn
        wq = np.ascontiguousarray(np.concatenate(parts, axis=1))

        b2p = np.zeros(EXTB * BLK, dtype=np.float32)
        b2p[glo - lo : ghi - lo] = b2[glo:ghi]
        b2e = np.ascontiguousarray(b2p.reshape(EXTB, BLK).T)

        B = None
        for b in range(SEGL, N, SEGL):
            if lo < b < hi:
                B = b
        ext_pos = lo + np.arange(EXTB)[None, :] * BLK + np.arange(BLK)[:, None]
        out_pos = out0 + np.arange(OUTB)[None, :] * BLK + np.arange(BLK)[:, None]
        if B is None:
            seg = min(out0 // SEGL, NSIG - 1)
            tl = tr = tiles[seg]
            ml = np.ones((BLK, EXTB), np.float32)
            mr = np.zeros((BLK, EXTB), np.float32)
            oml = np.ones((BLK, OUTB), np.float32)
            omr = np.zeros((BLK, OUTB), np.float32)
        else:
            tl = tiles[B // SEGL - 1]
            tr = tiles[B // SEGL]
            ml = (ext_pos < B).astype(np.float32)
            mr = (ext_pos >= B).astype(np.float32)
            oml = (out_pos < B).astype(np.float32)
            omr = (out_pos >= B).astype(np.float32)

        in_maps.append(
            {
                "wb": wb,
                "wq": wq,
                "hb": hb_in,
                "hq": hq_in,
                "b2e": b2e,
                "tl": np.ascontiguousarray(tl).astype(np.float16),
                "tr": np.ascontiguousarray(tr).astype(np.float16),
                "ml": ml,
                "mr": mr,
                "oml": oml,
                "omr": omr,
            }
        )
        meta.append((out0, k * PER - out0))
    return in_maps, meta


def _assemble(results, meta):
    full = np.empty(N, dtype=np.float32)
    for k in range(NCORES):
        arr = results[k]["out"]                         # [128, OUTB]
        flat = np.ascontiguousarray(arr.T).reshape(-1)  # pos out0 + i
        off = meta[k][1]
        full[k * PER : (k + 1) * PER] = flat[off : off + PER]
    return full


def run_with_results(inputs: dict, dt: str | None = None, trace: bool = False):
    args = {k: np.asarray(v, dtype=np.float32) for k, v in inputs.items()}
    in_maps, meta = _prep_inputs(
        args["x"], args["W_sig"], args["b_sig"], args["W1"], args["b1"],
        args["W2"], args["b2"],
    )
    nc = _build_nc()
    res = run_bass_kernel_spmd(
        nc, in_maps, core_ids=list(range(NCORES)), trace=trace
    )
    return _assemble(res.results, meta), res


def kernel(**inputs) -> np.ndarray:
    out, _ = run_with_results(inputs)
    return out


# revision 9
# speedup vs baseline: 1.3596x; 1.0045x over previous
"""Trainium2 Bass kernel for nn_Decoder (ragged_sequence).

Computes: sigmas = x@W_sig + b_sig; h = swish(x@W1 + b1); y = h@W2 + b2;
then per-segment gaussian smoothing (5 equal segments of 20000, window
10000, sigma ~ 200) of y, concatenated.

Strategy (8 NeuronCores, SPMD, full I/O):
  - Host computes the tiny parts (sigmas, h, gaussian windows) in numpy.
  - Output vector N=100000 is sharded over 8 cores (12500 each, rounded
    out to 99 blocks of 128). Each core GEMMs its slice of W2 columns
    (plus +-5 blocks of halo, zero-padded at the global edges) against
    h, producing y in a [128, cols] block layout (partition = pos%128).
  - Mixed precision, h-magnitude sorted: the 128 rows of W2 with the
    largest |h_k| are kept in fp16; the remaining 384 rows are quantized
    to fp8 e3m4 (x32 scale folded into the h operand). Quantization
    noise ~ sqrt(tail h^2 fraction)*1.3% ~ 0.45 %.
  - W2 slices are pre-tiled on the host into [128, X] DRAM tensors so
    each chunk is ONE contiguous-row 2D DMA descriptor (descriptor
    issue on the Sync engine costs ~630ns each; the baseline's 40
    descriptors serialized the stream). fp16 stream on the sync queue,
    fp8 stream on the scalar queue, constants on gpsimd.
  - The gaussian conv is applied as 11 shifted Toeplitz 128x128 matmuls
    accumulated in PSUM (window truncated to +-~767 taps, rel err
    ~4e-4). Segment boundaries are handled with left/right input masks
    + left/right tap tiles + an output select (host-built data, one
    uniform SPMD program for all cores).
"""

import os
from contextlib import ExitStack

import numpy as np

import ml_dtypes

import concourse.bass as bass
import concourse.mybir as mybir
import concourse.tile as tile
from concourse import bacc
from concourse.bass_utils import run_bass_kernel_spmd

# ---------------------------------------------------------------- constants
D = 128
H = 512
N = 100000
NSIG = 5
WIN = 10000          # reference window size
SEGL = 20000         # segment length
NCORES = 8
PER = N // NCORES    # 12500 outputs per core
BLK = 128
OUTB = 99            # output blocks per core (99*128 = 12672 >= 12500 + 84)
HB = 5               # halo blocks each side (+-640; sigma~200 -> err ~4e-4)
EXTB = OUTB + 2 * HB # 109 ext blocks of y per core
TAPB = 2 * HB + 1    # 11 Toeplitz shift tiles
CHUNKS = [8] * 13 + [5]  # W2 column-block chunks (sum = EXTB)
CONVSPLIT = 86           # conv pass 1 covers out cols [0, 86), pass 2 the rest
PASS1_AFTER = 11         # pass 1 runs after chunk 11 (cols 0..95 resident)
RB = 128             # rows (by |h| desc) kept in fp16
KQ = (H - RB) // BLK # 3 fp8 k-chunks
QS = 32.0            # fp8 weight scale (folded into the fp8-side h operand)

assert sum(CHUNKS) == EXTB

BSTART = [(k * PER) // BLK for k in range(NCORES)]

F8 = ml_dtypes.float8_e3m4

_CACHED_NC = {}


# ---------------------------------------------------------------- device IR
def _build_nc():
    """Build + compile the SPMD Bass kernel (same program for all cores)."""
    if "nc" in _CACHED_NC:
        return _CACHED_NC["nc"]
    f32 = mybir.dt.float32
    f16 = mybir.dt.float16
    f8e3 = mybir.dt.float8e3

    nc = bacc.Bacc(
        "TRN2",
        target_bir_lowering=False,
        debug=False,
        enable_asserts=False,
        num_devices=NCORES,
    )
    ins = {}

    def din(name, shape, dtt):
        ins[name] = nc.dram_tensor(name, shape, dtt, kind="ExternalInput").ap()

    din("wb", [BLK, EXTB * BLK], f16)           # fp16 W2 rows, col-tiled
    din("wq", [BLK, KQ * EXTB * BLK], f8e3)     # fp8 W2 rows, chunk-major
    din("hb", [BLK, 1], f16)
    din("hq", [BLK, KQ], f16)
    din("b2e", [BLK, EXTB], f32)
    din("tl", [BLK, TAPB * BLK], f16)
    din("tr", [BLK, TAPB * BLK], f16)
    din("ml", [BLK, EXTB], f32)
    din("mr", [BLK, EXTB], f32)
    din("oml", [BLK, OUTB], f32)
    din("omr", [BLK, OUTB], f32)
    out_ap = nc.dram_tensor("out", [BLK, OUTB], f32, kind="ExternalOutput").ap()

    with tile.TileContext(nc) as tc:
        with ExitStack() as ctx:
            const = ctx.enter_context(tc.tile_pool(name="const", bufs=1))
            wpool = ctx.enter_context(tc.tile_pool(name="w", bufs=6))
            sbp = ctx.enter_context(tc.tile_pool(name="sb", bufs=1))
            pgp = ctx.enter_context(tc.tile_pool(name="pg", bufs=3, space="PSUM"))
            pcp = ctx.enter_context(tc.tile_pool(name="pc", bufs=1, space="PSUM"))

            # The sync queue carries ONLY the fp16 W2 stream (+ taps
            # mid-stream) so its first descriptor issues immediately; the
            # scalar queue leads with the tiny h tiles (they gate the
            # first matmul) then carries the fp8 stream.  b2/ml/mr ride
            # the slow gpsimd software queue — they are only read by the
            # vector epilogues, which have slack.
            hb_sb = const.tile([BLK, 1], f16)
            nc.scalar.dma_start(hb_sb[:], ins["hb"][:])
            hq_sb = const.tile([BLK, KQ], f16)
            nc.scalar.dma_start(hq_sb[:], ins["hq"][:])
            b2_sb = const.tile([BLK, EXTB], f32)
            nc.gpsimd.dma_start(b2_sb[:], ins["b2e"][:])
            ml_sb = const.tile([BLK, EXTB], f32)
            nc.gpsimd.dma_start(ml_sb[:], ins["ml"][:])
            mr_sb = const.tile([BLK, EXTB], f32)
            nc.gpsimd.dma_start(mr_sb[:], ins["mr"][:])
            tl_sb = const.tile([BLK, TAPB * BLK], f16)
            tr_sb = const.tile([BLK, TAPB * BLK], f16)
            oml_sb = const.tile([BLK, OUTB], f32)
            omr_sb = const.tile([BLK, OUTB], f32)

            s_sb = sbp.tile([BLK, EXTB], f32)
            sl_sb = sbp.tile([BLK, EXTB], f16)
            sr_sb = sbp.tile([BLK, EXTB], f16)

            # ---- GEMM: y[c*128+q] = sum_k h[k] * W2[k, c*128+q]  (+ b2)
            # conv pass helper: out cols [r0, r1) as TAPB shifted
            # Toeplitz matmuls (left + right tap sets), then the output
            # boundary select and a partial writeback
            pA = pcp.tile([BLK, OUTB], f32, tag="pA")
            pB = pcp.tile([BLK, OUTB], f32, tag="pB")
            t1 = sbp.tile([BLK, OUTB], f32)
            t2 = sbp.tile([BLK, OUTB], f32)
            o_sb = sbp.tile([BLK, OUTB], f32)

            def conv_pass(r0, r1):
                for ei in range(TAPB):
                    nc.tensor.matmul(
                        pA[:, r0:r1],
                        lhsT=tl_sb[:, ei * BLK : (ei + 1) * BLK],
                        rhs=sl_sb[:, r0 + ei : r1 + ei],
                        start=(ei == 0),
                        stop=(ei == TAPB - 1),
                    )
                for ei in range(TAPB):
                    nc.tensor.matmul(
                        pB[:, r0:r1],
                        lhsT=tr_sb[:, ei * BLK : (ei + 1) * BLK],
                        rhs=sr_sb[:, r0 + ei : r1 + ei],
                        start=(ei == 0),
                        stop=(ei == TAPB - 1),
                    )
                nc.vector.tensor_mul(t1[:, r0:r1], pA[:, r0:r1], oml_sb[:, r0:r1])
                nc.vector.tensor_mul(t2[:, r0:r1], pB[:, r0:r1], omr_sb[:, r0:r1])
                nc.vector.tensor_add(o_sb[:, r0:r1], t1[:, r0:r1], t2[:, r0:r1])
                nc.scalar.dma_start(out_ap[:, r0:r1], o_sb[:, r0:r1])

            c0 = 0
            for ci, cbn in enumerate(CHUNKS):
                wbt = wpool.tile([BLK, cbn * BLK], f16, tag="wb")
                nc.sync.dma_start(
                    wbt[:], ins["wb"][:, c0 * BLK : (c0 + cbn) * BLK]
                )
                wqt = wpool.tile([BLK, KQ * cbn * BLK], f8e3, tag="wq")
                nc.scalar.dma_start(
                    wqt[:],
                    ins["wq"][:, KQ * c0 * BLK : KQ * (c0 + cbn) * BLK],
                )
                # interleave the conv constants into the hardware-queue
                # stream so they arrive well before the conv passes
                if ci == 3:
                    nc.sync.dma_start(tl_sb[:], ins["tl"][:])
                elif ci == 4:
                    nc.sync.dma_start(tr_sb[:], ins["tr"][:])
                elif ci == 2:
                    nc.scalar.dma_start(oml_sb[:], ins["oml"][:])
                    nc.scalar.dma_start(omr_sb[:], ins["omr"][:])
                psum_g = pgp.tile([BLK, cbn], f32, tag="pg")
                for cb in range(cbn):
                    nc.tensor.matmul(
                        psum_g[:, cb : cb + 1],
                        lhsT=wbt[:, cb * BLK : (cb + 1) * BLK],
                        rhs=hb_sb[:],
                        start=True,
                        stop=False,
                    )
                    for kc in range(KQ):
                        nc.tensor.matmul(
                            psum_g[:, cb : cb + 1],
                            lhsT=wqt[:, (kc * cbn + cb) * BLK : (kc * cbn + cb + 1) * BLK],
                            rhs=hq_sb[:, kc : kc + 1],
                            start=False,
                            stop=(kc == KQ - 1),
                        )
                # finalize this chunk's S columns (bias + boundary masks)
                sl = slice(c0, c0 + cbn)
                nc.vector.tensor_add(s_sb[:, sl], psum_g[:], b2_sb[:, sl])
                nc.vector.tensor_mul(sl_sb[:, sl], s_sb[:, sl], ml_sb[:, sl])
                nc.vector.tensor_mul(sr_sb[:, sl], s_sb[:, sl], mr_sb[:, sl])
                c0 += cbn
                if ci == PASS1_AFTER:
                    conv_pass(0, CONVSPLIT)

            conv_pass(CONVSPLIT, OUTB)

    nc.compile()
    _CACHED_NC["nc"] = nc
    return nc


# ---------------------------------------------------------------- host prep
def _prep_inputs(x, W_sig, b_sig, W1, b1, W2, b2):
    f64 = np.float64

    # tiny head + MLP hidden layer on host
    sig = x.astype(f64) @ W_sig.astype(f64) + b_sig.astype(f64)       # [5]
    pre = x.astype(f64) @ W1.astype(f64) + b1.astype(f64)             # [512]
    h = pre / (1.0 + np.exp(-pre))                                    # swish

    # mixed-precision split: biggest |h| rows stay fp16, rest fp8 e3m4
    ordr = np.argsort(-np.abs(h), kind="stable")
    hb_in = np.ascontiguousarray(h[ordr[:RB]].reshape(RB, 1)).astype(np.float16)
    hq_in = np.ascontiguousarray(
        (h[ordr[RB:]] / QS).reshape(KQ, BLK).T
    ).astype(np.float16)
    W2b = np.ascontiguousarray(W2[ordr[:RB], :]).astype(np.float16)   # [128,N]
    W2q = (np.ascontiguousarray(W2[ordr[RB:], :]) * np.float32(QS)).astype(F8)

    # normalized gaussian taps per segment: G_s(m) = exp(-m^2/2s^2)/Z_s
    # (Z over the full reference window t=0..9999 centered at 5000)
    t = np.arange(WIN, dtype=f64)
    Z = np.exp(-((t[None, :] - WIN / 2) ** 2) / (2 * sig[:, None] ** 2)).sum(axis=1)

    p = np.arange(BLK)[:, None]
    q = np.arange(BLK)[None, :]
    e = np.arange(-HB, HB + 1)[:, None, None]
    m = e * BLK + p[None] - q[None] + 1                               # [11,128,128]
    tiles = []
    for s in range(NSIG):
        g = np.exp(-(m.astype(f64) ** 2) / (2 * sig[s] ** 2)) / Z[s]
        tiles.append(
            np.ascontiguousarray(g.transpose(1, 0, 2)).reshape(BLK, -1)
        )

    in_maps = []
    meta = []
    for k in range(NCORES):
        lo = (BSTART[k] - HB) * BLK
        hi = lo + EXTB * BLK
        out0 = BSTART[k] * BLK
        glo, ghi = max(lo, 0), min(hi, N)

        wb = np.zeros((BLK, EXTB * BLK), dtype=np.float16)
        wb[:, glo - lo : ghi - lo] = W2b[:, glo:ghi]
        wqf = np.zeros((KQ * BLK, EXTB * BLK), dtype=F8)
        wqf[:, glo - lo : ghi - lo] = W2q[:, glo:ghi]
        # chunk-major fp8 layout: per chunk [kc][cb][q] so each chunk is
        # one contiguous-row DMA descriptor
        parts = []
        c0 = 0
        for cbn in CHUNKS:
            blockcols = wqf[:, c0 * BLK : (c0 + cbn) * BLK]
            parts.append(
                blockcols.reshape(KQ, BLK, cbn * BLK)
                .transpose(1, 0, 2)
                .reshape(BLK, KQ * cbn * BLK)
            )
            c0 += cbn
        wq = np.ascontiguousarray(np.concatenate(parts, axis=1))

        b2p = np.zeros(EXTB * BLK, dtype=np.float32)
        b2p[glo - lo : ghi - lo] = b2[glo:ghi]
        b2e = np.ascontiguousarray(b2p.reshape(EXTB, BLK).T)

        B = None
        for b in range(SEGL, N, SEGL):
            if lo < b < hi:
                B = b
        ext_pos = lo + np.arange(EXTB)[None, :] * BLK + np.arange(BLK)[:, None]
        out_pos = out0 + np.arange(OUTB)[None, :] * BLK + np.arange(BLK)[:, None]
        if B is None:
            seg = min(out0 // SEGL, NSIG - 1)
            tl = tr = tiles[seg]
            ml = np.ones((BLK, EXTB), np.float32)
            mr = np.zeros((BLK, EXTB), np.float32)
            oml = np.ones((BLK, OUTB), np.float32)
            omr = np.zeros((BLK, OUTB), np.float32)
        else:
            tl = tiles[B // SEGL - 1]
            tr = tiles[B // SEGL]
            ml = (ext_pos < B).astype(np.float32)
            mr = (ext_pos >= B).astype(np.float32)
            oml = (out_pos < B).astype(np.float32)
            omr = (out_pos >= B).astype(np.float32)

        in_maps.append(
            {
                "wb": wb,
                "wq": wq,
                "hb": hb_in,
                "hq": hq_in,
                "b2e": b2e,
                "tl": np.ascontiguousarray(tl).astype(np.float16),
                "tr": np.ascontiguousarray(tr).astype(np.float16),
                "ml": ml,
                "mr": mr,
                "oml": oml,
                "omr": omr,
            }
        )
        meta.append((out0, k * PER - out0))
    return in_maps, meta


def _assemble(results, meta):
    full = np.empty(N, dtype=np.float32)
    for k in range(NCORES):
        arr = results[k]["out"]                         # [128, OUTB]
        flat = np.ascontiguousarray(arr.T).reshape(-1)  # pos out0 + i
        off = meta[k][1]
        full[k * PER : (k + 1) * PER] = flat[off : off + PER]
    return full


def run_with_results(inputs: dict, dt: str | None = None, trace: bool = False):
    args = {k: np.asarray(v, dtype=np.float32) for k, v in inputs.items()}
    in_maps, meta = _prep_inputs(
        args["x"], args["W_sig"], args["b_sig"], args["W1"], args["b1"],
        args["W2"], args["b2"],
    )
    nc = _build_nc()
    res = run_bass_kernel_spmd(
        nc, in_maps, core_ids=list(range(NCORES)), trace=trace
    )
    return _assemble(res.results, meta), res


def kernel(**inputs) -> np.ndarray:
    out, _ = run_with_results(inputs)
    return out


# revision 17
# speedup vs baseline: 1.3939x; 1.0252x over previous
"""Trainium2 Bass kernel for nn_Decoder (ragged_sequence).

Computes: sigmas = x@W_sig + b_sig; h = swish(x@W1 + b1); y = h@W2 + b2;
then per-segment gaussian smoothing (5 equal segments of 20000, window
10000, sigma ~ 200) of y, concatenated.

Strategy (8 NeuronCores, SPMD, full I/O):
  - Host computes the tiny parts (sigmas, h, gaussian windows) in numpy.
  - Output vector N=100000 is sharded over 8 cores (12500 each, rounded
    out to 99 blocks of 128). Each core GEMMs its slice of W2 columns
    (plus +-5 blocks of halo, zero-padded at the global edges) against
    h, producing y in a [128, cols] block layout (partition = pos%128).
  - Mixed precision, h-magnitude sorted: the 128 rows of W2 with the
    largest |h_k| are kept in fp16; the remaining 384 rows are quantized
    to fp8 e3m4 (x32 scale folded into the h operand). Quantization
    noise ~ sqrt(tail h^2 fraction)*1.3% ~ 0.45 %.
  - W2 slices are pre-tiled on the host into [128, X] DRAM tensors so
    each chunk is ONE contiguous-row 2D DMA descriptor (descriptor
    issue on the Sync engine costs ~630ns each; the baseline's 40
    descriptors serialized the stream). fp16 stream on the sync queue,
    fp8 stream on the scalar queue, constants on gpsimd.
  - The gaussian conv is applied as 11 shifted Toeplitz 128x128 matmuls
    accumulated in PSUM (window truncated to +-~767 taps, rel err
    ~4e-4). Segment boundaries are handled with left/right input masks
    + left/right tap tiles + an output select (host-built data, one
    uniform SPMD program for all cores).
"""

import os
from contextlib import ExitStack

import numpy as np

import ml_dtypes

import concourse.bass as bass
import concourse.mybir as mybir
import concourse.tile as tile
from concourse import bacc
from concourse.bass_utils import run_bass_kernel_spmd

# ---------------------------------------------------------------- constants
D = 128
H = 512
N = 100000
NSIG = 5
WIN = 10000          # reference window size
SEGL = 20000         # segment length
NCORES = 8
PER = N // NCORES    # 12500 outputs per core
BLK = 128
OUTB = 99            # output blocks per core (99*128 = 12672 >= 12500 + 84)
HB = 5               # halo blocks each side (+-640; sigma~200 -> err ~4e-4)
EXTB = OUTB + 2 * HB # 109 ext blocks of y per core
TAPB = 2 * HB + 1    # 11 Toeplitz shift tiles
CHUNKS = [8] * 13 + [5]  # W2 column-block chunks (sum = EXTB)
# conv passes: (after_chunk, col_lo, col_hi) -- pass i runs right after
# that chunk's epilogue, when S cols [0, cum) are resident (col <= cum-TAPB)
CONVPASS = [(9, 0, 70), (12, 70, 94), (13, 94, OUTB)]
RB = 128             # rows (by |h| desc) kept in fp16
KQ = (H - RB) // BLK # 3 fp8 k-chunks
QS = 32.0            # fp8 weight scale (folded into the fp8-side h operand)

assert sum(CHUNKS) == EXTB

BSTART = [(k * PER) // BLK for k in range(NCORES)]

F8 = ml_dtypes.float8_e3m4

_CACHED_NC = {}


# ---------------------------------------------------------------- device IR
def _build_nc():
    """Build + compile the SPMD Bass kernel (same program for all cores)."""
    if "nc" in _CACHED_NC:
        return _CACHED_NC["nc"]
    f32 = mybir.dt.float32
    f16 = mybir.dt.float16
    f8e3 = mybir.dt.float8e3

    nc = bacc.Bacc(
        "TRN2",
        target_bir_lowering=False,
        debug=False,
        enable_asserts=False,
        num_devices=NCORES,
    )
    ins = {}

    def din(name, shape, dtt):
        ins[name] = nc.dram_tensor(name, shape, dtt, kind="ExternalInput").ap()

    din("wb", [BLK, EXTB * BLK], f16)           # fp16 W2 rows (+b2), col-tiled
    din("wq", [BLK, KQ * EXTB * BLK], f8e3)     # fp8 W2 rows, chunk-major
    din("hb", [BLK, 1], f16)
    din("hq", [BLK, KQ], f16)
    din("tl", [BLK, TAPB * BLK], f16)
    din("tr", [BLK, TAPB * BLK], f16)
    din("ml", [BLK, EXTB], f32)
    din("mr", [BLK, EXTB], f32)
    din("oml", [BLK, OUTB], f32)
    din("omr", [BLK, OUTB], f32)
    out_ap = nc.dram_tensor("out", [BLK, OUTB], f32, kind="ExternalOutput").ap()

    with tile.TileContext(nc) as tc:
        with ExitStack() as ctx:
            const = ctx.enter_context(tc.tile_pool(name="const", bufs=1))
            wpool = ctx.enter_context(tc.tile_pool(name="w", bufs=12))
            sbp = ctx.enter_context(tc.tile_pool(name="sb", bufs=1))
            pgp = ctx.enter_context(tc.tile_pool(name="pg", bufs=2, space="PSUM"))
            pcp = ctx.enter_context(tc.tile_pool(name="pc", bufs=1, space="PSUM"))

            # The sync queue carries the fp16 W2 stream, the scalar queue
            # the fp8 stream.  Tiny constants lead each queue (h gates the
            # first matmul, ml/mr the first epilogue); taps + output masks
            # are interleaved mid-stream, before the conv passes need them.
            hb_sb = const.tile([BLK, 1], f16)
            nc.scalar.dma_start(hb_sb[:], ins["hb"][:])
            hq_sb = const.tile([BLK, KQ], f16)
            nc.scalar.dma_start(hq_sb[:], ins["hq"][:])
            ml_sb = const.tile([BLK, EXTB], f32)
            nc.sync.dma_start(ml_sb[:], ins["ml"][:])
            mr_sb = const.tile([BLK, EXTB], f32)
            nc.scalar.dma_start(mr_sb[:], ins["mr"][:])
            tl_sb = const.tile([BLK, TAPB * BLK], f16)
            tr_sb = const.tile([BLK, TAPB * BLK], f16)
            oml_sb = const.tile([BLK, OUTB], f32)
            omr_sb = const.tile([BLK, OUTB], f32)

            sl_sb = sbp.tile([BLK, EXTB], f16)
            sr_sb = sbp.tile([BLK, EXTB], f16)

            # ---- GEMM: y[c*128+q] = sum_k h[k] * W2[k, c*128+q]
            # (b2 is folded in as wb's last row with h-coefficient 1)
            # conv pass: out cols [r0, r1) as TAPB shifted Toeplitz
            # matmuls (left + right tap sets) into pass-private PSUM
            # tiles, then the output boundary select + partial writeback
            t1 = sbp.tile([BLK, OUTB], f32)
            t2 = sbp.tile([BLK, OUTB], f32)
            o_sb = sbp.tile([BLK, OUTB], f32)

            def conv_pass(pi, r0, r1):
                pA = pcp.tile([BLK, r1 - r0], f32, tag=f"pA{pi}")
                pB = pcp.tile([BLK, r1 - r0], f32, tag=f"pB{pi}")
                for ei in range(TAPB):
                    nc.tensor.matmul(
                        pA[:, :],
                        lhsT=tl_sb[:, ei * BLK : (ei + 1) * BLK],
                        rhs=sl_sb[:, r0 + ei : r1 + ei],
                        start=(ei == 0),
                        stop=(ei == TAPB - 1),
                    )
                for ei in range(TAPB):
                    nc.tensor.matmul(
                        pB[:, :],
                        lhsT=tr_sb[:, ei * BLK : (ei + 1) * BLK],
                        rhs=sr_sb[:, r0 + ei : r1 + ei],
                        start=(ei == 0),
                        stop=(ei == TAPB - 1),
                    )
                nc.vector.tensor_mul(t1[:, r0:r1], pA[:, :], oml_sb[:, r0:r1])
                nc.vector.tensor_mul(t2[:, r0:r1], pB[:, :], omr_sb[:, r0:r1])
                nc.vector.tensor_add(o_sb[:, r0:r1], t1[:, r0:r1], t2[:, r0:r1])
                nc.scalar.dma_start(out_ap[:, r0:r1], o_sb[:, r0:r1])

            passes = list(CONVPASS)
            c0 = 0
            for ci, cbn in enumerate(CHUNKS):
                wbt = wpool.tile([BLK, cbn * BLK], f16, tag="wb")
                nc.sync.dma_start(
                    wbt[:], ins["wb"][:, c0 * BLK : (c0 + cbn) * BLK]
                )
                wqt = wpool.tile([BLK, KQ * cbn * BLK], f8e3, tag="wq")
                nc.scalar.dma_start(
                    wqt[:],
                    ins["wq"][:, KQ * c0 * BLK : KQ * (c0 + cbn) * BLK],
                )
                # interleave the conv constants into the hardware-queue
                # stream so they arrive well before the conv passes
                if ci == 3:
                    nc.sync.dma_start(tl_sb[:], ins["tl"][:])
                elif ci == 4:
                    nc.sync.dma_start(tr_sb[:], ins["tr"][:])
                elif ci == 2:
                    nc.scalar.dma_start(oml_sb[:], ins["oml"][:])
                    nc.scalar.dma_start(omr_sb[:], ins["omr"][:])
                psum_g = pgp.tile([BLK, cbn], f32, tag="pg")
                for cb in range(cbn):
                    nc.tensor.matmul(
                        psum_g[:, cb : cb + 1],
                        lhsT=wbt[:, cb * BLK : (cb + 1) * BLK],
                        rhs=hb_sb[:],
                        start=True,
                        stop=False,
                    )
                    for kc in range(KQ):
                        nc.tensor.matmul(
                            psum_g[:, cb : cb + 1],
                            lhsT=wqt[:, (kc * cbn + cb) * BLK : (kc * cbn + cb + 1) * BLK],
                            rhs=hq_sb[:, kc : kc + 1],
                            start=False,
                            stop=(kc == KQ - 1),
                        )
                # finalize this chunk's S columns (boundary masks)
                sl = slice(c0, c0 + cbn)
                nc.vector.tensor_mul(sl_sb[:, sl], psum_g[:], ml_sb[:, sl])
                nc.vector.tensor_mul(sr_sb[:, sl], psum_g[:], mr_sb[:, sl])
                c0 += cbn
                while passes and passes[0][0] == ci:
                    _, r0, r1 = passes.pop(0)
                    conv_pass(len(CONVPASS) - len(passes) - 1, r0, r1)

    nc.compile()
    _CACHED_NC["nc"] = nc
    return nc


# ---------------------------------------------------------------- host prep
def _prep_inputs(x, W_sig, b_sig, W1, b1, W2, b2):
    f64 = np.float64

    # tiny head + MLP hidden layer on host
    sig = x.astype(f64) @ W_sig.astype(f64) + b_sig.astype(f64)       # [5]
    pre = x.astype(f64) @ W1.astype(f64) + b1.astype(f64)             # [512]
    h = pre / (1.0 + np.exp(-pre))                                    # swish

    # mixed-precision split: biggest |h| rows stay fp16, rest fp8 e3m4;
    # b2 is folded into the top fp16 row (scaled by 1/h_top, exact for
    # b2 == 0) so the device never needs a separate bias add
    ordr = np.argsort(-np.abs(h), kind="stable")
    hb_in = np.ascontiguousarray(h[ordr[:RB]].reshape(RB, 1)).astype(np.float16)
    hq_in = np.ascontiguousarray(
        (h[ordr[RB:]] / QS).reshape(KQ, BLK).T
    ).astype(np.float16)
    W2bf = W2[ordr[:RB], :].astype(f64)
    W2bf[0] += b2.astype(f64) / float(hb_in[0, 0])
    W2b = W2bf.astype(np.float16)                                     # [128,N]
    W2q = (np.ascontiguousarray(W2[ordr[RB:], :]) * np.float32(QS)).astype(F8)

    # normalized gaussian taps per segment: G_s(m) = exp(-m^2/2s^2)/Z_s
    # (Z over the full reference window t=0..9999 centered at 5000)
    t = np.arange(WIN, dtype=f64)
    Z = np.exp(-((t[None, :] - WIN / 2) ** 2) / (2 * sig[:, None] ** 2)).sum(axis=1)

    p = np.arange(BLK)[:, None]
    q = np.arange(BLK)[None, :]
    e = np.arange(-HB, HB + 1)[:, None, None]
    m = e * BLK + p[None] - q[None] + 1                               # [11,128,128]
    tiles = []
    for s in range(NSIG):
        g = np.exp(-(m.astype(f64) ** 2) / (2 * sig[s] ** 2)) / Z[s]
        tiles.append(
            np.ascontiguousarray(g.transpose(1, 0, 2)).reshape(BLK, -1)
        )

    in_maps = []
    meta = []
    for k in range(NCORES):
        lo = (BSTART[k] - HB) * BLK
        hi = lo + EXTB * BLK
        out0 = BSTART[k] * BLK
        glo, ghi = max(lo, 0), min(hi, N)

        wb = np.zeros((BLK, EXTB * BLK), dtype=np.float16)
        wb[:, glo - lo : ghi - lo] = W2b[:, glo:ghi]
        wqf = np.zeros((KQ * BLK, EXTB * BLK), dtype=F8)
        wqf[:, glo - lo : ghi - lo] = W2q[:, glo:ghi]
        # chunk-major fp8 layout: per chunk [kc][cb][q] so each chunk is
        # one contiguous-row DMA descriptor
        parts = []
        c0 = 0
        for cbn in CHUNKS:
            blockcols = wqf[:, c0 * BLK : (c0 + cbn) * BLK]
            parts.append(
                blockcols.reshape(KQ, BLK, cbn * BLK)
                .transpose(1, 0, 2)
                .reshape(BLK, KQ * cbn * BLK)
            )
            c0 += cbn
        wq = np.ascontiguousarray(np.concatenate(parts, axis=1))

        B = None
        for b in range(SEGL, N, SEGL):
            if lo < b < hi:
                B = b
        ext_pos = lo + np.arange(EXTB)[None, :] * BLK + np.arange(BLK)[:, None]
        out_pos = out0 + np.arange(OUTB)[None, :] * BLK + np.arange(BLK)[:, None]
        if B is None:
            seg = min(out0 // SEGL, NSIG - 1)
            tl = tr = tiles[seg]
            ml = np.ones((BLK, EXTB), np.float32)
            mr = np.zeros((BLK, EXTB), np.float32)
            oml = np.ones((BLK, OUTB), np.float32)
            omr = np.zeros((BLK, OUTB), np.float32)
        else:
            tl = tiles[B // SEGL - 1]
            tr = tiles[B // SEGL]
            ml = (ext_pos < B).astype(np.float32)
            mr = (ext_pos >= B).astype(np.float32)
            oml = (out_pos < B).astype(np.float32)
            omr = (out_pos >= B).astype(np.float32)

        in_maps.append(
            {
                "wb": wb,
                "wq": wq,
                "hb": hb_in,
                "hq": hq_in,
                "tl": np.ascontiguousarray(tl).astype(np.float16),
                "tr": np.ascontiguousarray(tr).astype(np.float16),
                "ml": ml,
                "mr": mr,
                "oml": oml,
                "omr": omr,
            }
        )
        meta.append((out0, k * PER - out0))
    return in_maps, meta


def _assemble(results, meta):
    full = np.empty(N, dtype=np.float32)
    for k in range(NCORES):
        arr = results[k]["out"]                         # [128, OUTB]
        flat = np.ascontiguousarray(arr.T).reshape(-1)  # pos out0 + i
        off = meta[k][1]
        full[k * PER : (k + 1) * PER] = flat[off : off + PER]
    return full


def run_with_results(inputs: dict, dt: str | None = None, trace: bool = False):
    args = {k: np.asarray(v, dtype=np.float32) for k, v in inputs.items()}
    in_maps, meta = _prep_inputs(
        args["x"], args["W_sig"], args["b_sig"], args["W1"], args["b1"],
        args["W2"], args["b2"],
    )
    nc = _build_nc()
    res = run_bass_kernel_spmd(
        nc, in_maps, core_ids=list(range(NCORES)), trace=trace
    )
    return _assemble(res.results, meta), res


def kernel(**inputs) -> np.ndarray:
    out, _ = run_with_results(inputs)
    return out
